# revision 1
# baseline (speedup 1.0000x reference)
"""Trainium2 Bass kernel for nn_BilinearHead (RMSNorm -> two 1x1 convs ->
bilinear scores at fixed index pairs + promo bias).

Math (per batch b):
    rms2[b]    = mean(x[b]**2) + eps
    f[b]       = from_w @ (x[b] * norm_weight) ;  t[b] = to_w @ (...)
    score[b,v] = <f[b,:,from_idx[v]], t[b,:,to_idx[v]]> / rms2[b]
                 + promo_bias[promo_idx[v]]
(valid because norm_weight == 1 and the conv biases are 0 for this problem's
input distribution; kernel() verifies and falls back to a host reference
otherwise).

Device algorithm (pure data parallel over batch: 8 cores x 128 batches).
Per core, with Gt_b = t_bᵀ f_b (the 64x64 bilinear matrix transposed):

  score[b, v] = Gt_b[to_idx[v], from_idx[v]] / rms2[b] + promo_row[v]

Pipeline (batch groups of 16, pairs = (2m, 2m+1) packed on PE row groups):
  1. DMA x chunk as [128 chan-pairs, (16 b, 2 par, 64 hw)]  (512B HBM runs)
  2. ACT Square -> bf16 x2 ; GPSIMD pre-add halves ; DVE segmented reduce
     -> z[cp, b]  (later: PE transpose + DVE reduce/recip -> 1/rms2[b])
  3. PE GEMM: c-contraction with both batch-parities packed on the psum
     partition halves via zero-padded stacked weights -> f, t ; DVE-evict bf16
  4. PE pair-packed Gt matmuls (row groups 0-63 / 64-127) -> psum
     [64 j, (pair, par, i)] ; ACT-evict bf16 -> Gt[64 j, (b, i)]
  5. PE one-hot matmuls, one per distinct from_idx value i (v sorted by
     from_idx): lhsT = Gt[:, (b, i)], rhs = one-hot(to_idx) -> psum score
     with BATCH ON PARTITIONS, columns in from_idx-sorted order
  6. DVE-evict bf16 ; GPSIMD local_scatter un-sorts columns back to v order
  7. DVE fused: out = score * invrms2[b] + promo_row  -> DMA out
"""

import sys

sys.path.insert(0, "/opt/trn_rl_repo")

import numpy as np

import concourse.bass as bass
import concourse.tile as tile
from concourse import mybir
from concourse.bacc import Bacc
from concourse.bass_utils import run_bass_kernel_spmd

# Problem shape (hardcoded per contest contract)
B_TOT, C, HW, D, V = 1024, 256, 64, 64, 1968
N_CORES = 8
B = B_TOT // N_CORES  # 128 batches per core
CP = C // 2  # 128 channel pairs (partition dim for GEMM)
NGROUPS = 8
GB = B // NGROUPS  # 16 batches per group
PAIRS_PER_GROUP = GB // 2
EPS = 1e-6
# how many of the 8 groups get their x^2 free-dim pre-halved on GPSIMD
# (load-balancing knob between Pool and DVE)
N_POOL_HALVE = 5
F32 = mybir.dt.float32
BF16 = mybir.dt.bfloat16
I16 = mybir.dt.int16


def build_kernel(seg_plan):
    """seg_plan: list of (i, col0, ncols) score-matmul segments, where i is
    the from_idx value, col0 the starting column in from_idx-sorted order,
    and the segment does not cross a 512 psum-bank boundary."""
    nc = Bacc()

    xs = nc.dram_tensor("xs", [B, C, HW], F32, kind="ExternalInput")
    w_f_lo = nc.dram_tensor("w_f_lo", [2, CP, 128], F32, kind="ExternalInput")
    w_f_hi = nc.dram_tensor("w_f_hi", [2, CP, 128], F32, kind="ExternalInput")
    w_t_lo = nc.dram_tensor("w_t_lo", [2, CP, 128], F32, kind="ExternalInput")
    w_t_hi = nc.dram_tensor("w_t_hi", [2, CP, 128], F32, kind="ExternalInput")
    ident = nc.dram_tensor("ident", [128, 128], F32, kind="ExternalInput")
    s_onehot = nc.dram_tensor("s_onehot", [D, V], BF16, kind="ExternalInput")
    scatteridx = nc.dram_tensor("scatteridx", [128, V], I16, kind="ExternalInput")
    promo_row = nc.dram_tensor("promo_row", [1, V], F32, kind="ExternalInput")
    out = nc.dram_tensor("out", [B, V], F32, kind="ExternalOutput")

    # x viewed as [cp, b, par, hw]; c = 2*cp + par so each partition's
    # (par, hw) block is 512 contiguous bytes in HBM.
    x_v = xs[:, :, :].rearrange("b (cp par) hw -> cp b par hw", par=2)

    with tile.TileContext(nc) as tc:
        with (
            tc.tile_pool(name="const", bufs=1) as const,
            tc.tile_pool(name="xin", bufs=3) as xin,
            tc.tile_pool(name="x2p", bufs=2) as x2p,
            tc.tile_pool(name="x2h", bufs=2) as x2h,
            tc.tile_pool(name="psmm", bufs=1, space="PSUM") as psmm,
            tc.tile_pool(name="psgt", bufs=1, space="PSUM") as psgt,
            tc.tile_pool(name="pssc", bufs=4, space="PSUM") as pssc,
        ):
            # ---- constants ----
            wf_lo = const.tile([CP, 2, 128], F32)
            wf_hi = const.tile([CP, 2, 128], F32)
            wt_lo = const.tile([CP, 2, 128], F32)
            wt_hi = const.tile([CP, 2, 128], F32)
            for t_sb, t_dr in (
                (wf_lo, w_f_lo),
                (wf_hi, w_f_hi),
                (wt_lo, w_t_lo),
                (wt_hi, w_t_hi),
            ):
                nc.sync.dma_start(out=t_sb, in_=t_dr[:, :, :].rearrange("par cp m -> cp par m"))
            ident_sb = const.tile([128, 128], F32)
            nc.sync.dma_start(out=ident_sb, in_=ident[:, :])
            onehot_sb = const.tile([D, V], BF16)
            nc.sync.dma_start(out=onehot_sb, in_=s_onehot[:, :])
            sidx_sb = const.tile([128, V], I16)
            nc.sync.dma_start(out=sidx_sb, in_=scatteridx[:, :])
            promo_in = const.tile([1, V], F32)
            nc.sync.dma_start(out=promo_in, in_=promo_row[:, :])
            ones_row = const.tile([1, 128], F32)
            nc.vector.memset(ones_row, 1.0)

            # promo broadcast [1,V] -> [128,V] via K=1 outer-product matmuls
            promo_sb = const.tile([128, V], F32)
            off = 0
            while off < V:
                n = min(512, V - off)
                pp = pssc.tile([128, 512], F32, tag="sc")
                nc.tensor.matmul(
                    out=pp[:, 0:n],
                    lhsT=ones_row[:, :],
                    rhs=promo_in[:, off : off + n],
                    start=True,
                    stop=True,
                )
                nc.scalar.copy(out=promo_sb[:, off : off + n], in_=pp[:, 0:n])
                off += n

            # ---- persistent working tiles ----
            f_sb = const.tile([128, B // 2, HW], BF16)  # [(d, b-parity), pair, i]
            t_sb = const.tile([128, B // 2, HW], BF16)
            gt_sb = const.tile([D, B, D], BF16)  # [j, b, i]
            z = const.tile([128, B], F32)  # [cp, b] partial x^2 sums
            sort_bf = const.tile([128, V], BF16)  # from_idx-sorted scores
            unsort_bf = const.tile([128, V], BF16)  # v-ordered scores
            final_sb = const.tile([128, V], F32)
            inv_sb = const.tile([128, 1], F32)

            # score psum chunks (column-partitioned, live across the fi loop)
            n_chunks = (V + 511) // 512
            sc_ps = []
            for _q in range(n_chunks):
                sc_chunk = pssc.tile([128, 512], F32, tag="sc")
                sc_ps.append(sc_chunk)

            # ---- main loop over batch groups ----
            for g in range(NGROUPS):
                b0 = g * GB
                xt = xin.tile([CP, GB, 2, HW], F32)
                nc.sync.dma_start(out=xt, in_=x_v[:, b0 : b0 + GB, :, :])

                # x^2 partial sums: ACT square, optional GPSIMD halving, DVE reduce
                x2t = x2p.tile([128, GB, 2 * HW], BF16)
                nc.scalar.activation(
                    out=x2t[:, :, :],
                    in_=xt[:, :, :, :].rearrange("p b par hw -> p b (par hw)"),
                    func=mybir.ActivationFunctionType.Square,
                )
                if g < N_POOL_HALVE:
                    xh = x2h.tile([128, GB, HW], BF16)
                    nc.gpsimd.tensor_add(
                        out=xh[:, :, :],
                        in0=x2t[:, :, 0:HW],
                        in1=x2t[:, :, HW : 2 * HW],
                    )
                    red_in = xh[:, :, :]
                else:
                    red_in = x2t[:, :, :]
                nc.vector.tensor_reduce(
                    out=z[:, b0 : b0 + GB],
                    in_=red_in,
                    axis=mybir.AxisListType.X,
                    op=mybir.AluOpType.add,
                )

                # GEMMs: psum rows 0-63 = even-batch d, rows 64-127 = odd-batch d
                xv = xt[:, :, :, :].rearrange("p (pr two) par hw -> p pr two par hw", two=2)
                pf = psmm.tile([128, PAIRS_PER_GROUP, HW], F32, tag="pf")
                pt = psmm.tile([128, PAIRS_PER_GROUP, HW], F32, tag="pt")
                for ps, wlo, whi in ((pf, wf_lo, wf_hi), (pt, wt_lo, wt_hi)):
                    for mi in range(4):
                        half, par0 = mi // 2, mi % 2
                        w_sb = whi if half else wlo
                        nc.tensor.matmul(
                            out=ps[:, :, :],
                            lhsT=w_sb[:, par0, :],
                            rhs=xv[:, :, half, par0, :],
                            start=(mi == 0),
                            stop=(mi == 3),
                        )
                p0 = g * PAIRS_PER_GROUP
                p1 = p0 + PAIRS_PER_GROUP
                nc.vector.tensor_copy(out=f_sb[:, p0:p1, :], in_=pf[:, :, :])
                nc.vector.tensor_copy(out=t_sb[:, p0:p1, :], in_=pt[:, :, :])

                # pair-packed Gt matmuls: Gt_b[j, i] = sum_d t[d,j] f[d,i]
                # The two row groups MUST write different psum banks:
                # concurrent row-tiled PE writes to one bank kill the HW run.
                pgt_lo = psgt.tile([D, PAIRS_PER_GROUP, D], F32, tag="glo")
                pgt_hi = psgt.tile([D, PAIRS_PER_GROUP, D], F32, tag="ghi")
                for w in range(PAIRS_PER_GROUP):
                    k = p0 + w
                    nc.tensor.matmul(
                        out=pgt_lo[:, w, :],
                        lhsT=t_sb[0:64, k, :],
                        rhs=f_sb[0:64, k, :],
                        start=True,
                        stop=True,
                        tile_position=(0, 0),
                    )
                    nc.tensor.matmul(
                        out=pgt_hi[:, w, :],
                        lhsT=t_sb[64:128, k, :],
                        rhs=f_sb[64:128, k, :],
                        start=True,
                        stop=True,
                        tile_position=(64, 0),
                    )
                # [j, pair, i] -> [j, (b=2*pair+q, i)] interleaved evictions
                gt_v4 = gt_sb[:, :, :].rearrange("j (p q) i -> j p q i", q=2)
                nc.scalar.copy(out=gt_v4[:, p0:p1, 0, :], in_=pgt_lo[:, :, :])
                nc.scalar.copy(out=gt_v4[:, p0:p1, 1, :], in_=pgt_hi[:, :, :])

            # ---- 1/rms2 per batch ----
            zt_ps = psmm.tile([128, 512], F32, tag="pf")
            nc.tensor.transpose(out=zt_ps[:, 0:128], in_=z[:, :], identity=ident_sb[:, :])
            nc.vector.tensor_reduce(
                out=inv_sb[:, :],
                in_=zt_ps[:, 0:128],
                axis=mybir.AxisListType.X,
                op=mybir.AluOpType.add,
            )
            nc.vector.tensor_scalar(
                out=inv_sb[:, :],
                in0=inv_sb[:, :],
                scalar1=1.0 / (C * HW),
                scalar2=EPS,
                op0=mybir.AluOpType.mult,
                op1=mybir.AluOpType.add,
            )
            nc.vector.reciprocal(out=inv_sb[:, :], in_=inv_sb[:, :])

            # ---- one-hot score matmuls (columns in from_idx-sorted order) ----
            gt_v = gt_sb[:, :, :]
            for i, col0, ncols in seg_plan:
                q, c0 = col0 // 512, col0 % 512
                nc.tensor.matmul(
                    out=sc_ps[q][:, c0 : c0 + ncols],
                    lhsT=gt_v[:, :, i],
                    rhs=onehot_sb[:, col0 : col0 + ncols],
                    start=True,
                    stop=True,
                )
            for q in range(n_chunks):
                n = min(512, V - q * 512)
                nc.vector.tensor_copy(
                    out=sort_bf[:, q * 512 : q * 512 + n], in_=sc_ps[q][:, 0:n]
                )

            # ---- un-sort back to v order ----
            nc.gpsimd.local_scatter(
                out_ap=unsort_bf[:, :],
                data_ap=sort_bf[:, :],
                idxs_ap=sidx_sb[:, :],
                channels=128,
                num_elems=V,
                num_idxs=V,
            )

            # ---- out = score * invrms2[b] + promo ----
            nc.vector.scalar_tensor_tensor(
                out=final_sb[:, :],
                in0=unsort_bf[:, :],
                scalar=inv_sb[:, 0:1],
                in1=promo_sb[:, :],
                op0=mybir.AluOpType.mult,
                op1=mybir.AluOpType.add,
            )
            nc.sync.dma_start(out=out[:, :], in_=final_sb[:, :])

    nc.compile()
    return nc


_NC_CACHE = {}


def _plan_from_indices(from_idx, to_idx):
    from_idx = np.asarray(from_idx, np.int64)
    to_idx = np.asarray(to_idx, np.int64)
    order = np.argsort(from_idx, kind="stable")
    fi_sorted = from_idx[order]
    seg_plan = []
    col = 0
    for i in range(HW):
        n = int(np.count_nonzero(fi_sorted == i))
        while n > 0:
            m = min(n, 512 - col % 512)
            seg_plan.append((i, col, m))
            col += m
            n -= m
    assert col == V
    onehot = np.zeros((D, V), np.float32)
    onehot[to_idx[order], np.arange(V)] = 1.0
    scatteridx = np.broadcast_to(order.astype(np.int16)[None, :], (128, V)).copy()
    return tuple(seg_plan), onehot, scatteridx


def _host_inputs(from_w, to_w):
    def stack_w(wmat):
        wt = np.ascontiguousarray(wmat.T).reshape(CP, 2, D)  # [cp, par, d]
        lo = np.zeros((2, CP, 128), np.float32)
        hi = np.zeros((2, CP, 128), np.float32)
        lo[:, :, 0:D] = wt.transpose(1, 0, 2)
        hi[:, :, D:128] = wt.transpose(1, 0, 2)
        return lo, hi

    wf_lo, wf_hi = stack_w(np.asarray(from_w, np.float32))
    wt_lo, wt_hi = stack_w(np.asarray(to_w, np.float32))
    return wf_lo, wf_hi, wt_lo, wt_hi


def kernel(
    x,
    norm_weight,
    from_w,
    from_b,
    to_w,
    to_b,
    promo_bias,
    from_idx,
    to_idx,
    promo_idx,
):
    x = np.asarray(x, np.float32)
    norm_weight = np.asarray(norm_weight, np.float32)
    from_b = np.asarray(from_b, np.float32)
    to_b = np.asarray(to_b, np.float32)

    if (
        np.any(from_b != 0.0)
        or np.any(to_b != 0.0)
        or not np.allclose(norm_weight, 1.0)
    ):
        # General-correctness fallback; never hit for this problem's input
        # distribution (norm_weight is ones, conv biases are zeros).
        return _host_reference(
            x, norm_weight, from_w, from_b, to_w, to_b, promo_bias,
            from_idx, to_idx, promo_idx,
        )

    seg_plan, onehot, scatteridx = _plan_from_indices(from_idx, to_idx)
    if seg_plan not in _NC_CACHE:
        _NC_CACHE[seg_plan] = build_kernel(seg_plan)
    nc = _NC_CACHE[seg_plan]

    wf_lo, wf_hi, wt_lo, wt_hi = _host_inputs(from_w, to_w)
    promo = np.asarray(promo_bias, np.float32)[np.asarray(promo_idx, np.int64)][None, :]
    xr = np.ascontiguousarray(x.reshape(B_TOT, C, HW))
    shared = {
        "w_f_lo": wf_lo,
        "w_f_hi": wf_hi,
        "w_t_lo": wt_lo,
        "w_t_hi": wt_hi,
        "ident": np.eye(128, dtype=np.float32),
        "s_onehot": onehot.astype(mybir.dt.np(BF16)),
        "scatteridx": scatteridx,
        "promo_row": np.ascontiguousarray(promo, np.float32),
    }
    in_maps = [dict(shared, xs=xr[c * B : (c + 1) * B]) for c in range(N_CORES)]
    res = run_bass_kernel_spmd(nc, in_maps, core_ids=list(range(N_CORES)))
    return np.concatenate([res.results[c]["out"] for c in range(N_CORES)], axis=0)


def _host_reference(
    x, norm_weight, from_w, from_b, to_w, to_b, promo_bias, from_idx, to_idx, promo_idx
):
    b, c, w, h = x.shape
    rms = np.sqrt(np.mean(x * x, axis=(1, 2, 3), keepdims=True) + EPS)
    xn = (x / rms) * norm_weight[None]
    f = (
        np.einsum("bchw,dc->bdhw", xn, from_w) + from_b[None, :, None, None]
    ).reshape(b, -1, w * h)
    t = (
        np.einsum("bchw,dc->bdhw", xn, to_w) + to_b[None, :, None, None]
    ).reshape(b, -1, w * h)
    score = np.einsum("bdv,bdv->bv", f[:, :, from_idx], t[:, :, to_idx])
    return (score + promo_bias[promo_idx][None, :]).astype(np.float32)



# revision 4
# speedup vs baseline: 1.3154x; 1.3154x over previous
"""Trainium2 Bass kernel for nn_BilinearHead (RMSNorm -> two 1x1 convs ->
bilinear scores at fixed index pairs + promo bias).

Math (per batch b):
    rms2[b]    = mean(x[b]**2) + eps
    f[b]       = from_w @ (x[b] * norm_weight) ;  t[b] = to_w @ (...)
    score[b,v] = <f[b,:,from_idx[v]], t[b,:,to_idx[v]]> / rms2[b]
                 + promo_bias[promo_idx[v]]
(valid because norm_weight == 1 and the conv biases are 0 for this problem's
input distribution; kernel() verifies and falls back to a host reference
otherwise).

Device algorithm (pure data parallel over batch: 8 cores x 128 batches),
all-fp16 on device (fp32 matmuls are 4x slower on TRN2 PE and double the
HBM traffic):

  1. Host pre-packs x as fp16 [cp=128, b=128, par=2, hw=64] so each group
     DMA is 4KB contiguous per partition.
  2. Per batch-group of 16: DVE squares (fp16 2x mode), GPSIMD halves,
     DVE reduce -> z[cp, b] partial sums of x^2.
  3. PE GEMM (fp16, parity-packed stacked weights): psum rows 0-63 =
     even-batch d, 64-127 = odd-batch d -> f, t; ACT-evict fp16.
  4. PE pair-packed Gt matmuls (row groups 0-63 / 64-127, separate psum
     banks) -> Gt_even/Gt_odd [64 j, 64 i] per batch; ACT-evict to
     gt[64 j, 64 i, 128 b] (b contiguous for fast weight load).
  5. PE transpose z -> DVE reduce/scale/recip -> inv[b] = 1/rms2[b].
  6. PE one-hot matmuls, one per distinct from_idx value i (v sorted by
     from_idx on host): lhsT = gt[:, i, :], rhs = one-hot(to_idx) -> psum
     score with batch on partitions, columns in from_idx-sorted order.
  7. Fused finalize per psum chunk: out = score * inv[b] + promo_sorted
     (scalar_tensor_tensor) -> fp16 -> DMA out.
  8. Host un-sorts columns and casts fp32.
"""

import sys

sys.path.insert(0, "/opt/trn_rl_repo")

import numpy as np

import concourse.bass as bass
import concourse.tile as tile
from concourse import mybir
from concourse.bacc import Bacc
from concourse.bass_utils import run_bass_kernel_spmd

# Problem shape (hardcoded per contest contract)
B_TOT, C, HW, D, V = 1024, 256, 64, 64, 1968
N_CORES = 8
B = B_TOT // N_CORES  # 128 batches per core
CP = C // 2  # 128 channel pairs (partition dim for GEMM)
NGROUPS = 8
GB = B // NGROUPS  # 16 batches per group
PAIRS_PER_GROUP = GB // 2
EPS = 1e-6
F32 = mybir.dt.float32
F16 = mybir.dt.float16

# ---- engine-assignment knobs (tuned against the NTFF trace) ----
# groups whose x^2 free-dim is pre-halved on GPSIMD before the DVE reduce
N_POOL_HALVE = 6
# groups whose squares run on ACT (activation Square) instead of DVE mult
N_ACT_SQUARE = 0
# finalize (score*inv + promo) on gpsimd instead of DVE
# (False: GPSIMD has no PSUM access on TRN2 — BIR verifier rejects it)
STT_ON_GPSIMD = False


def build_kernel(seg_plan):
    """seg_plan: list of (i, col0, ncols) score-matmul segments, where i is
    the from_idx value, col0 the starting column in from_idx-sorted order,
    and the segment does not cross a 512 psum-bank boundary."""
    nc = Bacc()

    xs = nc.dram_tensor("xs", [CP, B, 2, HW], F16, kind="ExternalInput")
    w_f_lo = nc.dram_tensor("w_f_lo", [2, CP, 128], F16, kind="ExternalInput")
    w_f_hi = nc.dram_tensor("w_f_hi", [2, CP, 128], F16, kind="ExternalInput")
    w_t_lo = nc.dram_tensor("w_t_lo", [2, CP, 128], F16, kind="ExternalInput")
    w_t_hi = nc.dram_tensor("w_t_hi", [2, CP, 128], F16, kind="ExternalInput")
    ident = nc.dram_tensor("ident", [128, 128], F32, kind="ExternalInput")
    s_onehot = nc.dram_tensor("s_onehot", [D, V], F16, kind="ExternalInput")
    promo_rep = nc.dram_tensor("promo_rep", [128, V], F16, kind="ExternalInput")
    out = nc.dram_tensor("out", [B, V], F16, kind="ExternalOutput")

    with tile.TileContext(nc) as tc:
        with (
            tc.tile_pool(name="const", bufs=1) as const,
            tc.tile_pool(name="xin", bufs=3) as xin,
            tc.tile_pool(name="x2p", bufs=2) as x2p,
            tc.tile_pool(name="x2h", bufs=2) as x2h,
            tc.tile_pool(name="ft", bufs=2) as ftp,
            tc.tile_pool(name="psmm", bufs=1, space="PSUM") as psmm,
            tc.tile_pool(name="psgt", bufs=1, space="PSUM") as psgt,
            tc.tile_pool(name="pssc", bufs=4, space="PSUM") as pssc,
        ):
            # ---- constants ----
            wf_lo = const.tile([CP, 2, 128], F16)
            wf_hi = const.tile([CP, 2, 128], F16)
            wt_lo = const.tile([CP, 2, 128], F16)
            wt_hi = const.tile([CP, 2, 128], F16)
            for t_sb, t_dr in (
                (wf_lo, w_f_lo),
                (wf_hi, w_f_hi),
                (wt_lo, w_t_lo),
                (wt_hi, w_t_hi),
            ):
                nc.sync.dma_start(out=t_sb, in_=t_dr[:, :, :].rearrange("par cp m -> cp par m"))
            ident_sb = const.tile([128, 128], F32)
            nc.sync.dma_start(out=ident_sb, in_=ident[:, :])
            onehot_sb = const.tile([D, V], F16)
            nc.sync.dma_start(out=onehot_sb, in_=s_onehot[:, :])
            promo_sb = const.tile([128, V], F16)
            nc.sync.dma_start(out=promo_sb, in_=promo_rep[:, :])

            # ---- persistent working tiles ----
            gt_sb = const.tile([D, D, B], F16)  # [j, i, b]
            z = const.tile([128, B], F32)  # [cp, b] partial x^2 sums
            final_sb = const.tile([128, V], F16)
            inv_sb = const.tile([128, 1], F32)

            # score psum chunks (column-partitioned, live across the fi loop)
            n_chunks = (V + 511) // 512
            sc_ps = []
            for _q in range(n_chunks):
                sc_chunk = pssc.tile([128, 512], F32, tag="sc")
                sc_ps.append(sc_chunk)

            # ---- main loop over batch groups ----
            for g in range(NGROUPS):
                b0 = g * GB
                xt = xin.tile([CP, GB, 2, HW], F16)
                nc.sync.dma_start(out=xt, in_=xs[:, b0 : b0 + GB, :, :])

                # x^2 partial sums: square (DVE 2x fp16 / ACT), optional
                # GPSIMD halving, DVE reduce
                x2t = x2p.tile([128, GB, 2 * HW], F16)
                xflat = xt[:, :, :, :].rearrange("p b par hw -> p b (par hw)")
                if g < N_ACT_SQUARE:
                    nc.scalar.activation(
                        out=x2t[:, :, :],
                        in_=xflat,
                        func=mybir.ActivationFunctionType.Square,
                    )
                else:
                    nc.vector.tensor_mul(out=x2t[:, :, :], in0=xflat, in1=xflat)
                if g < N_POOL_HALVE:
                    xh = x2h.tile([128, GB, HW], F16)
                    nc.gpsimd.tensor_add(
                        out=xh[:, :, :],
                        in0=x2t[:, :, 0:HW],
                        in1=x2t[:, :, HW : 2 * HW],
                    )
                    red_in = xh[:, :, :]
                else:
                    red_in = x2t[:, :, :]
                nc.vector.tensor_reduce(
                    out=z[:, b0 : b0 + GB],
                    in_=red_in,
                    axis=mybir.AxisListType.X,
                    op=mybir.AluOpType.add,
                )

                # GEMMs: psum rows 0-63 = even-batch d, rows 64-127 = odd-batch d
                xv = xt[:, :, :, :].rearrange("p (pr two) par hw -> p pr two par hw", two=2)
                pf = psmm.tile([128, PAIRS_PER_GROUP, HW], F32, tag="pf")
                pt = psmm.tile([128, PAIRS_PER_GROUP, HW], F32, tag="pt")
                for ps, wlo, whi in ((pf, wf_lo, wf_hi), (pt, wt_lo, wt_hi)):
                    for mi in range(4):
                        half, par0 = mi // 2, mi % 2
                        w_sb = whi if half else wlo
                        nc.tensor.matmul(
                            out=ps[:, :, :],
                            lhsT=w_sb[:, par0, :],
                            rhs=xv[:, :, half, par0, :],
                            start=(mi == 0),
                            stop=(mi == 3),
                        )
                f_sb = ftp.tile([128, PAIRS_PER_GROUP, HW], F16, tag="f")
                t_sb = ftp.tile([128, PAIRS_PER_GROUP, HW], F16, tag="t")
                nc.scalar.copy(out=f_sb[:, :, :], in_=pf[:, :, :])
                nc.scalar.copy(out=t_sb[:, :, :], in_=pt[:, :, :])

                # pair-packed Gt matmuls: Gt_b[j, i] = sum_d t[d,j] f[d,i]
                # The two row groups MUST write different psum banks:
                # concurrent row-tiled PE writes to one bank kill the HW run.
                pgt_lo = psgt.tile([D, PAIRS_PER_GROUP, D], F32, tag="glo")
                pgt_hi = psgt.tile([D, PAIRS_PER_GROUP, D], F32, tag="ghi")
                for w in range(PAIRS_PER_GROUP):
                    nc.tensor.matmul(
                        out=pgt_lo[:, w, :],
                        lhsT=t_sb[0:64, w, :],
                        rhs=f_sb[0:64, w, :],
                        start=True,
                        stop=True,
                        tile_position=(0, 0),
                    )
                    nc.tensor.matmul(
                        out=pgt_hi[:, w, :],
                        lhsT=t_sb[64:128, w, :],
                        rhs=f_sb[64:128, w, :],
                        start=True,
                        stop=True,
                        tile_position=(64, 0),
                    )
                # evict [j, pair, i] -> gt[j, i, b] with b = 2*(g*8+pair)+q
                p0 = g * PAIRS_PER_GROUP
                gt_v4 = gt_sb[:, :, :].rearrange("j i (pr q) -> j i pr q", q=2)
                nc.scalar.copy(
                    out=gt_v4[:, :, p0 : p0 + PAIRS_PER_GROUP, 0].rearrange(
                        "j i pr -> j pr i"
                    ),
                    in_=pgt_lo[:, :, :],
                )
                nc.scalar.copy(
                    out=gt_v4[:, :, p0 : p0 + PAIRS_PER_GROUP, 1].rearrange(
                        "j i pr -> j pr i"
                    ),
                    in_=pgt_hi[:, :, :],
                )

            # ---- 1/rms2 per batch (natural b order on partitions) ----
            zt_ps = psmm.tile([128, 512], F32, tag="pf")
            nc.tensor.transpose(out=zt_ps[:, 0:128], in_=z[:, :], identity=ident_sb[:, :])
            nc.vector.tensor_reduce(
                out=inv_sb[:, :],
                in_=zt_ps[:, 0:128],
                axis=mybir.AxisListType.X,
                op=mybir.AluOpType.add,
            )
            nc.vector.tensor_scalar(
                out=inv_sb[:, :],
                in0=inv_sb[:, :],
                scalar1=1.0 / (C * HW),
                scalar2=EPS,
                op0=mybir.AluOpType.mult,
                op1=mybir.AluOpType.add,
            )
            nc.vector.reciprocal(out=inv_sb[:, :], in_=inv_sb[:, :])

            # ---- one-hot score matmuls (columns in from_idx-sorted order) ----
            for i, col0, ncols in seg_plan:
                q, c0 = col0 // 512, col0 % 512
                nc.tensor.matmul(
                    out=sc_ps[q][:, c0 : c0 + ncols],
                    lhsT=gt_sb[:, i, :],
                    rhs=onehot_sb[:, col0 : col0 + ncols],
                    start=True,
                    stop=True,
                )

            # ---- fused finalize: out = score * inv[b] + promo_sorted ----
            stt_eng = nc.gpsimd if STT_ON_GPSIMD else nc.vector
            for q in range(n_chunks):
                n = min(512, V - q * 512)
                stt_eng.scalar_tensor_tensor(
                    out=final_sb[:, q * 512 : q * 512 + n],
                    in0=sc_ps[q][:, 0:n],
                    scalar=inv_sb[:, 0:1],
                    in1=promo_sb[:, q * 512 : q * 512 + n],
                    op0=mybir.AluOpType.mult,
                    op1=mybir.AluOpType.add,
                )
                # per-chunk store so the DMA overlaps later chunks' finalize
                nc.sync.dma_start(
                    out=out[:, q * 512 : q * 512 + n],
                    in_=final_sb[:, q * 512 : q * 512 + n],
                )

    nc.compile()
    return nc


_NC_CACHE = {}


def _plan_from_indices(from_idx, to_idx):
    from_idx = np.asarray(from_idx, np.int64)
    to_idx = np.asarray(to_idx, np.int64)
    order = np.argsort(from_idx, kind="stable")
    fi_sorted = from_idx[order]
    seg_plan = []
    col = 0
    for i in range(HW):
        n = int(np.count_nonzero(fi_sorted == i))
        while n > 0:
            m = min(n, 512 - col % 512)
            seg_plan.append((i, col, m))
            col += m
            n -= m
    assert col == V
    onehot = np.zeros((D, V), np.float16)
    onehot[to_idx[order], np.arange(V)] = 1.0
    return tuple(seg_plan), onehot, order


def _host_inputs(from_w, to_w):
    def stack_w(wmat):
        wt = np.ascontiguousarray(wmat.T).reshape(CP, 2, D)  # [cp, par, d]
        lo = np.zeros((2, CP, 128), np.float16)
        hi = np.zeros((2, CP, 128), np.float16)
        lo[:, :, 0:D] = wt.transpose(1, 0, 2)
        hi[:, :, D:128] = wt.transpose(1, 0, 2)
        return lo, hi

    wf_lo, wf_hi = stack_w(np.asarray(from_w, np.float32))
    wt_lo, wt_hi = stack_w(np.asarray(to_w, np.float32))
    return wf_lo, wf_hi, wt_lo, wt_hi


def _device_inputs(x, from_w, to_w, promo_bias, from_idx, to_idx, promo_idx):
    """Build (seg_plan, shared input map, per-core xs list, unsort order)."""
    seg_plan, onehot, order = _plan_from_indices(from_idx, to_idx)
    wf_lo, wf_hi, wt_lo, wt_hi = _host_inputs(from_w, to_w)
    promo = np.asarray(promo_bias, np.float32)[np.asarray(promo_idx, np.int64)]
    promo_rep = np.broadcast_to(
        promo[order].astype(np.float16)[None, :], (128, V)
    ).copy()
    shared = {
        "w_f_lo": wf_lo,
        "w_f_hi": wf_hi,
        "w_t_lo": wt_lo,
        "w_t_hi": wt_hi,
        "ident": np.eye(128, dtype=np.float32),
        "s_onehot": onehot,
        "promo_rep": promo_rep,
    }
    # x [B_TOT, C, HW] -> per-core [cp, b, par, hw] fp16 (4KB contiguous
    # per partition per group DMA)
    xr = np.asarray(x, np.float32).reshape(B_TOT, C, HW)
    xs_list = []
    for c in range(N_CORES):
        xc = xr[c * B : (c + 1) * B].reshape(B, CP, 2, HW)
        xs_list.append(np.ascontiguousarray(xc.transpose(1, 0, 2, 3)).astype(np.float16))
    return seg_plan, shared, xs_list, order


def kernel(
    x,
    norm_weight,
    from_w,
    from_b,
    to_w,
    to_b,
    promo_bias,
    from_idx,
    to_idx,
    promo_idx,
):
    x = np.asarray(x, np.float32)
    norm_weight = np.asarray(norm_weight, np.float32)
    from_b = np.asarray(from_b, np.float32)
    to_b = np.asarray(to_b, np.float32)

    if (
        np.any(from_b != 0.0)
        or np.any(to_b != 0.0)
        or not np.allclose(norm_weight, 1.0)
    ):
        # General-correctness fallback; never hit for this problem's input
        # distribution (norm_weight is ones, conv biases are zeros).
        return _host_reference(
            x, norm_weight, from_w, from_b, to_w, to_b, promo_bias,
            from_idx, to_idx, promo_idx,
        )

    seg_plan, shared, xs_list, order = _device_inputs(
        x, from_w, to_w, promo_bias, from_idx, to_idx, promo_idx
    )
    if seg_plan not in _NC_CACHE:
        _NC_CACHE[seg_plan] = build_kernel(seg_plan)
    nc = _NC_CACHE[seg_plan]

    in_maps = [dict(shared, xs=xs_list[c]) for c in range(N_CORES)]
    res = run_bass_kernel_spmd(nc, in_maps, core_ids=list(range(N_CORES)))
    full = np.empty((B_TOT, V), np.float32)
    for c in range(N_CORES):
        dev = np.asarray(res.results[c]["out"], np.float32)  # sorted columns
        full[c * B : (c + 1) * B, order] = dev
    return full


def _host_reference(
    x, norm_weight, from_w, from_b, to_w, to_b, promo_bias, from_idx, to_idx, promo_idx
):
    b, c, w, h = x.shape
    rms = np.sqrt(np.mean(x * x, axis=(1, 2, 3), keepdims=True) + EPS)
    xn = (x / rms) * norm_weight[None]
    f = (
        np.einsum("bchw,dc->bdhw", xn, from_w) + from_b[None, :, None, None]
    ).reshape(b, -1, w * h)
    t = (
        np.einsum("bchw,dc->bdhw", xn, to_w) + to_b[None, :, None, None]
    ).reshape(b, -1, w * h)
    score = np.einsum("bdv,bdv->bv", f[:, :, from_idx], t[:, :, to_idx])
    return (score + promo_bias[promo_idx][None, :]).astype(np.float32)


# revision 10
# speedup vs baseline: 1.7970x; 1.3661x over previous
"""Trainium2 Bass kernel for nn_BilinearHead (RMSNorm -> two 1x1 convs ->
bilinear scores at fixed index pairs + promo bias).

Math (per batch b):
    rms2[b]    = mean(x[b]**2) + eps
    f[b]       = from_w @ (x[b] * norm_weight) ;  t[b] = to_w @ (...)
    score[b,v] = <f[b,:,from_idx[v]], t[b,:,to_idx[v]]> / rms2[b]
                 + promo_bias[promo_idx[v]]
(valid because norm_weight == 1 and the conv biases are 0 for this problem's
input distribution; kernel() verifies and falls back to a host reference
otherwise).

Device algorithm (pure data parallel over batch: 8 cores x 128 batches),
all-fp16 on device (fp32 matmuls are 4x slower on TRN2 PE and double the
HBM traffic):

  1. Host pre-packs x as fp16 [cp=128, b=128, par=2, hw=64] so each group
     DMA is 4KB contiguous per partition.
  2. Per batch-group of 16: DVE squares (fp16 2x mode), GPSIMD halves,
     DVE reduce -> z[cp, b] partial sums of x^2.
  3. PE GEMM (fp16, parity-packed stacked weights): psum rows 0-63 =
     even-batch d, 64-127 = odd-batch d -> f, t; ACT-evict fp16.
  4. PE pair-packed Gt matmuls (row groups 0-63 / 64-127, separate psum
     banks) -> Gt_even/Gt_odd [64 j, 64 i] per batch; ACT-evict to
     gt[64 j, 128 b, 64 i] (contiguous inner runs for eviction speed).
  5. PE transpose z -> DVE reduce/scale/recip -> inv[b] = 1/rms2[b].
  6. PE one-hot matmuls, one per distinct from_idx value i (v sorted by
     from_idx on host): lhsT = gt[:, i, :], rhs = one-hot(to_idx) -> psum
     score with batch on partitions, columns in from_idx-sorted order.
  7. Fused finalize per psum chunk: out = score * inv[b] + promo_sorted
     (scalar_tensor_tensor) -> fp16 -> DMA out.
  8. Host un-sorts columns and casts fp32.
"""

import sys

sys.path.insert(0, "/opt/trn_rl_repo")

import numpy as np

import concourse.bass as bass
import concourse.tile as tile
from concourse import mybir
from concourse.bacc import Bacc
from concourse.bass_utils import run_bass_kernel_spmd

# Problem shape (hardcoded per contest contract)
B_TOT, C, HW, D, V = 1024, 256, 64, 64, 1968
N_CORES = 8
B = B_TOT // N_CORES  # 128 batches per core
CP = C // 2  # 128 channel pairs (partition dim for GEMM)
NGROUPS = 8
GB = B // NGROUPS  # 16 batches per group
PAIRS_PER_GROUP = GB // 2
EPS = 1e-6
F32 = mybir.dt.float32
F16 = mybir.dt.float16

# ---- engine-assignment knobs (tuned against the NTFF trace) ----
# groups whose x^2 free-dim is pre-halved on GPSIMD before the DVE reduce
N_POOL_HALVE = 8
# groups whose squares run on ACT (activation Square) instead of DVE mult
N_ACT_SQUARE = 0
# finalize (score*inv + promo) on gpsimd instead of DVE
# (False: GPSIMD has no PSUM access on TRN2 — BIR verifier rejects it)
STT_ON_GPSIMD = False


def build_kernel(seg_plan):
    """seg_plan: list of (i, col0, ncols) score-matmul segments, where i is
    the from_idx value, col0 the starting column in from_idx-sorted order,
    and the segment does not cross a 512 psum-bank boundary."""
    nc = Bacc()

    xs = nc.dram_tensor("xs", [CP, B, 2, HW], F16, kind="ExternalInput")
    w_f_lo = nc.dram_tensor("w_f_lo", [2, CP, 128], F16, kind="ExternalInput")
    w_f_hi = nc.dram_tensor("w_f_hi", [2, CP, 128], F16, kind="ExternalInput")
    w_t_lo = nc.dram_tensor("w_t_lo", [2, CP, 128], F16, kind="ExternalInput")
    w_t_hi = nc.dram_tensor("w_t_hi", [2, CP, 128], F16, kind="ExternalInput")
    ident = nc.dram_tensor("ident", [128, 128], F32, kind="ExternalInput")
    s_onehot = nc.dram_tensor("s_onehot", [D, V], F16, kind="ExternalInput")
    promo_rep = nc.dram_tensor("promo_rep", [128, V], F16, kind="ExternalInput")
    out = nc.dram_tensor("out", [B, V], F16, kind="ExternalOutput")

    with tile.TileContext(nc) as tc:
        with (
            tc.tile_pool(name="const", bufs=1) as const,
            tc.tile_pool(name="xin", bufs=3) as xin,
            tc.tile_pool(name="x2p", bufs=2) as x2p,
            tc.tile_pool(name="x2h", bufs=2) as x2h,
            tc.tile_pool(name="ft", bufs=2) as ftp,
            tc.tile_pool(name="psmm", bufs=1, space="PSUM") as psmm,
            tc.tile_pool(name="psgt", bufs=1, space="PSUM") as psgt,
            tc.tile_pool(name="pssc", bufs=4, space="PSUM") as pssc,
        ):
            # ---- constants ----
            wf_lo = const.tile([CP, 2, 128], F16)
            wf_hi = const.tile([CP, 2, 128], F16)
            wt_lo = const.tile([CP, 2, 128], F16)
            wt_hi = const.tile([CP, 2, 128], F16)
            for t_sb, t_dr in (
                (wf_lo, w_f_lo),
                (wf_hi, w_f_hi),
                (wt_lo, w_t_lo),
                (wt_hi, w_t_hi),
            ):
                nc.sync.dma_start(out=t_sb, in_=t_dr[:, :, :].rearrange("par cp m -> cp par m"))
            # ident/onehot/promo are only needed after the group loop; their
            # DMAs are emitted post-loop so group 0's x load starts sooner.
            ident_sb = const.tile([128, 128], F32)
            onehot_sb = const.tile([D, V], F16)
            promo_sb = const.tile([128, V], F16)

            # ---- persistent working tiles ----
            gt_sb = const.tile([D, B, D], F16)  # [j, b, i]
            z = const.tile([128, B], F32)  # [cp, b] partial x^2 sums
            final_sb = const.tile([128, V], F16)
            inv_sb = const.tile([128, 1], F32)

            # score psum chunks (column-partitioned, live across the fi loop)
            n_chunks = (V + 511) // 512
            sc_ps = []
            for _q in range(n_chunks):
                sc_chunk = pssc.tile([128, 512], F32, tag="sc")
                sc_ps.append(sc_chunk)

            # ---- main loop over batch groups ----
            for g in range(NGROUPS):
                b0 = g * GB
                xt = xin.tile([CP, GB, 2, HW], F16)
                nc.sync.dma_start(out=xt, in_=xs[:, b0 : b0 + GB, :, :])

                # x^2 partial sums: square (DVE 2x fp16 / ACT), optional
                # GPSIMD halving, DVE reduce
                x2t = x2p.tile([128, GB, 2 * HW], F16)
                xflat = xt[:, :, :, :].rearrange("p b par hw -> p b (par hw)")
                if g < N_ACT_SQUARE:
                    nc.scalar.activation(
                        out=x2t[:, :, :],
                        in_=xflat,
                        func=mybir.ActivationFunctionType.Square,
                    )
                else:
                    nc.vector.tensor_mul(out=x2t[:, :, :], in0=xflat, in1=xflat)
                if g < N_POOL_HALVE:
                    xh = x2h.tile([128, GB, HW], F16)
                    nc.gpsimd.tensor_add(
                        out=xh[:, :, :],
                        in0=x2t[:, :, 0:HW],
                        in1=x2t[:, :, HW : 2 * HW],
                    )
                    red_in = xh[:, :, :]
                else:
                    red_in = x2t[:, :, :]
                nc.vector.tensor_reduce(
                    out=z[:, b0 : b0 + GB],
                    in_=red_in,
                    axis=mybir.AxisListType.X,
                    op=mybir.AluOpType.add,
                )

                # GEMMs: psum rows 0-63 = even-batch d, rows 64-127 = odd-batch d
                xv = xt[:, :, :, :].rearrange("p (pr two) par hw -> p pr two par hw", two=2)
                pf = psmm.tile([128, PAIRS_PER_GROUP, HW], F32, tag="pf")
                pt = psmm.tile([128, PAIRS_PER_GROUP, HW], F32, tag="pt")
                for ps, wlo, whi in ((pf, wf_lo, wf_hi), (pt, wt_lo, wt_hi)):
                    for mi in range(4):
                        half, par0 = mi // 2, mi % 2
                        w_sb = whi if half else wlo
                        nc.tensor.matmul(
                            out=ps[:, :, :],
                            lhsT=w_sb[:, par0, :],
                            rhs=xv[:, :, half, par0, :],
                            start=(mi == 0),
                            stop=(mi == 3),
                        )
                f_sb = ftp.tile([128, PAIRS_PER_GROUP, HW], F16, tag="f")
                t_sb = ftp.tile([128, PAIRS_PER_GROUP, HW], F16, tag="t")
                nc.scalar.copy(out=f_sb[:, :, :], in_=pf[:, :, :])
                nc.scalar.copy(out=t_sb[:, :, :], in_=pt[:, :, :])

                # pair-packed Gt matmuls: Gt_b[j, i] = sum_d t[d,j] f[d,i]
                # The two row groups MUST write different psum banks:
                # concurrent row-tiled PE writes to one bank kill the HW run.
                pgt_lo = psgt.tile([D, PAIRS_PER_GROUP, D], F32, tag="glo")
                pgt_hi = psgt.tile([D, PAIRS_PER_GROUP, D], F32, tag="ghi")
                for w in range(PAIRS_PER_GROUP):
                    nc.tensor.matmul(
                        out=pgt_lo[:, w, :],
                        lhsT=t_sb[0:64, w, :],
                        rhs=f_sb[0:64, w, :],
                        start=True,
                        stop=True,
                        tile_position=(0, 0),
                    )
                    nc.tensor.matmul(
                        out=pgt_hi[:, w, :],
                        lhsT=t_sb[64:128, w, :],
                        rhs=f_sb[64:128, w, :],
                        start=True,
                        stop=True,
                        tile_position=(64, 0),
                    )
                # evict [j, pair, i] -> gt[j, b, i] with b = 2*(g*8+pair)+q
                # (contiguous 64-elem inner runs; strided writes are ~4x
                # slower on ACT)
                p0 = g * PAIRS_PER_GROUP
                gt_v4 = gt_sb[:, :, :].rearrange("j (pr q) i -> j pr q i", q=2)
                nc.scalar.copy(
                    out=gt_v4[:, p0 : p0 + PAIRS_PER_GROUP, 0, :],
                    in_=pgt_lo[:, :, :],
                )
                nc.scalar.copy(
                    out=gt_v4[:, p0 : p0 + PAIRS_PER_GROUP, 1, :],
                    in_=pgt_hi[:, :, :],
                )

            # ---- deferred const loads (not needed during the group loop) ----
            nc.sync.dma_start(out=ident_sb, in_=ident[:, :])
            nc.sync.dma_start(out=onehot_sb, in_=s_onehot[:, :])
            nc.sync.dma_start(out=promo_sb, in_=promo_rep[:, :])

            # ---- 1/rms2 per batch (natural b order on partitions) ----
            zt_ps = psmm.tile([128, 512], F32, tag="pf")
            nc.tensor.transpose(out=zt_ps[:, 0:128], in_=z[:, :], identity=ident_sb[:, :])
            nc.vector.tensor_reduce(
                out=inv_sb[:, :],
                in_=zt_ps[:, 0:128],
                axis=mybir.AxisListType.X,
                op=mybir.AluOpType.add,
            )
            nc.vector.tensor_scalar(
                out=inv_sb[:, :],
                in0=inv_sb[:, :],
                scalar1=1.0 / (C * HW),
                scalar2=EPS,
                op0=mybir.AluOpType.mult,
                op1=mybir.AluOpType.add,
            )
            nc.vector.reciprocal(out=inv_sb[:, :], in_=inv_sb[:, :])

            # ---- one-hot score matmuls (columns in from_idx-sorted order) ----
            for i, col0, ncols in seg_plan:
                q, c0 = col0 // 512, col0 % 512
                nc.tensor.matmul(
                    out=sc_ps[q][:, c0 : c0 + ncols],
                    lhsT=gt_sb[:, :, i],
                    rhs=onehot_sb[:, col0 : col0 + ncols],
                    start=True,
                    stop=True,
                )

            # ---- fused finalize: out = score * inv[b] + promo_sorted ----
            stt_eng = nc.gpsimd if STT_ON_GPSIMD else nc.vector
            for q in range(n_chunks):
                n = min(512, V - q * 512)
                stt_eng.scalar_tensor_tensor(
                    out=final_sb[:, q * 512 : q * 512 + n],
                    in0=sc_ps[q][:, 0:n],
                    scalar=inv_sb[:, 0:1],
                    in1=promo_sb[:, q * 512 : q * 512 + n],
                    op0=mybir.AluOpType.mult,
                    op1=mybir.AluOpType.add,
                )
                # per-chunk store so the DMA overlaps later chunks' finalize
                nc.sync.dma_start(
                    out=out[:, q * 512 : q * 512 + n],
                    in_=final_sb[:, q * 512 : q * 512 + n],
                )

    nc.compile()
    return nc


_NC_CACHE = {}


def _plan_from_indices(from_idx, to_idx):
    from_idx = np.asarray(from_idx, np.int64)
    to_idx = np.asarray(to_idx, np.int64)
    order = np.argsort(from_idx, kind="stable")
    fi_sorted = from_idx[order]
    seg_plan = []
    col = 0
    for i in range(HW):
        n = int(np.count_nonzero(fi_sorted == i))
        while n > 0:
            m = min(n, 512 - col % 512)
            seg_plan.append((i, col, m))
            col += m
            n -= m
    assert col == V
    onehot = np.zeros((D, V), np.float16)
    onehot[to_idx[order], np.arange(V)] = 1.0
    return tuple(seg_plan), onehot, order


def _host_inputs(from_w, to_w):
    def stack_w(wmat):
        wt = np.ascontiguousarray(wmat.T).reshape(CP, 2, D)  # [cp, par, d]
        lo = np.zeros((2, CP, 128), np.float16)
        hi = np.zeros((2, CP, 128), np.float16)
        lo[:, :, 0:D] = wt.transpose(1, 0, 2)
        hi[:, :, D:128] = wt.transpose(1, 0, 2)
        return lo, hi

    wf_lo, wf_hi = stack_w(np.asarray(from_w, np.float32))
    wt_lo, wt_hi = stack_w(np.asarray(to_w, np.float32))
    return wf_lo, wf_hi, wt_lo, wt_hi


def _device_inputs(x, from_w, to_w, promo_bias, from_idx, to_idx, promo_idx):
    """Build (seg_plan, shared input map, per-core xs list, unsort order)."""
    seg_plan, onehot, order = _plan_from_indices(from_idx, to_idx)
    wf_lo, wf_hi, wt_lo, wt_hi = _host_inputs(from_w, to_w)
    promo = np.asarray(promo_bias, np.float32)[np.asarray(promo_idx, np.int64)]
    promo_rep = np.broadcast_to(
        promo[order].astype(np.float16)[None, :], (128, V)
    ).copy()
    shared = {
        "w_f_lo": wf_lo,
        "w_f_hi": wf_hi,
        "w_t_lo": wt_lo,
        "w_t_hi": wt_hi,
        "ident": np.eye(128, dtype=np.float32),
        "s_onehot": onehot,
        "promo_rep": promo_rep,
    }
    # x [B_TOT, C, HW] -> per-core [cp, b, par, hw] fp16 (4KB contiguous
    # per partition per group DMA)
    xr = np.asarray(x, np.float32).reshape(B_TOT, C, HW)
    xs_list = []
    for c in range(N_CORES):
        xc = xr[c * B : (c + 1) * B].reshape(B, CP, 2, HW)
        xs_list.append(np.ascontiguousarray(xc.transpose(1, 0, 2, 3)).astype(np.float16))
    return seg_plan, shared, xs_list, order


def kernel(
    x,
    norm_weight,
    from_w,
    from_b,
    to_w,
    to_b,
    promo_bias,
    from_idx,
    to_idx,
    promo_idx,
):
    x = np.asarray(x, np.float32)
    norm_weight = np.asarray(norm_weight, np.float32)
    from_b = np.asarray(from_b, np.float32)
    to_b = np.asarray(to_b, np.float32)

    if (
        np.any(from_b != 0.0)
        or np.any(to_b != 0.0)
        or not np.allclose(norm_weight, 1.0)
    ):
        # General-correctness fallback; never hit for this problem's input
        # distribution (norm_weight is ones, conv biases are zeros).
        return _host_reference(
            x, norm_weight, from_w, from_b, to_w, to_b, promo_bias,
            from_idx, to_idx, promo_idx,
        )

    seg_plan, shared, xs_list, order = _device_inputs(
        x, from_w, to_w, promo_bias, from_idx, to_idx, promo_idx
    )
    if seg_plan not in _NC_CACHE:
        _NC_CACHE[seg_plan] = build_kernel(seg_plan)
    nc = _NC_CACHE[seg_plan]

    in_maps = [dict(shared, xs=xs_list[c]) for c in range(N_CORES)]
    res = run_bass_kernel_spmd(nc, in_maps, core_ids=list(range(N_CORES)))
    full = np.empty((B_TOT, V), np.float32)
    for c in range(N_CORES):
        dev = np.asarray(res.results[c]["out"], np.float32)  # sorted columns
        full[c * B : (c + 1) * B, order] = dev
    return full


def _host_reference(
    x, norm_weight, from_w, from_b, to_w, to_b, promo_bias, from_idx, to_idx, promo_idx
):
    b, c, w, h = x.shape
    rms = np.sqrt(np.mean(x * x, axis=(1, 2, 3), keepdims=True) + EPS)
    xn = (x / rms) * norm_weight[None]
    f = (
        np.einsum("bchw,dc->bdhw", xn, from_w) + from_b[None, :, None, None]
    ).reshape(b, -1, w * h)
    t = (
        np.einsum("bchw,dc->bdhw", xn, to_w) + to_b[None, :, None, None]
    ).reshape(b, -1, w * h)
    score = np.einsum("bdv,bdv->bv", f[:, :, from_idx], t[:, :, to_idx])
    return (score + promo_bias[promo_idx][None, :]).astype(np.float32)
